# revision 8
# baseline (speedup 1.0000x reference)
"""CIN (xDeepFM) 3-layer kernel for Trainium2, 8-core data parallel. v5.

Math (per layer l, with IN = input viewed [F=64, n] and X = previous
activation [H, n], n = (b, d) flattened):
    pre[o, n] = sum_{h, f} Wl[o, h, f] * X[h, n] * IN[f, n]
    Xnext = relu(pre + bl);  out_l[o, b] = sum_d Xnext[o, (b, d)]

v5 changes vs v2 (190us):
  - 16 of the 64 shared L1/L2 f-slices move off the saturated DVE to the
    idle GPSIMD engine: one native tensor_mul per layer-chunk computes
    z[f48..63] = X * (IN/8) from an fp8 partition-replicated table
    (half the DMA bytes of bf16), writing fp8; the contraction for those
    slices runs as plain fp8 matmuls against 8x-scaled fp8 weights (the
    8/(1/8) scales cancel in PSUM, so fp8 weight values stay in e4m3's
    normal range). Cuts DVE z-work by 25% and table DMA by ~12%.
  - Layer 0 keeps the v2 symmetric mod-64 diagonal tiling (K 4096 ->
    2112) on the DVE+bf16-table route.
"""

import numpy as np
import ml_dtypes

import concourse.bass as bass
import concourse.bacc as bacc
import concourse.tile as tile
import concourse.mybir as mybir
from concourse.bass_utils import run_bass_kernel_spmd

BF16 = ml_dtypes.bfloat16
E4M3 = ml_dtypes.float8_e4m3fn

B, F, D = 512, 64, 32
NCORES = 8
BL = B // NCORES          # 64 batches per core
N = BL * D                # 2048 columns per core
CH = 512                  # chunk width (columns)
NCH = N // CH             # 4 chunks
O = 128                   # out channels per layer
GRP = 8                   # table rows per slot tile
NT0 = 16                  # full layer-0 k-tiles (plus one K=64 tail)
NSL = 6                   # DVE-route slots for layers 1/2 (f 0..47)
NBF = NSL * GRP           # DVE-route f-slices = 48
NPO = F - NBF             # Pool-route f-slices (f 48..63) = 16
SW = 8.0                  # fp8 weight prescale (tables scaled by 1/SW)
bf16 = mybir.dt.bfloat16
f32 = mybir.dt.float32
fp8 = mybir.dt.float8e4

_cache = {}


def _build_program(bench_repeat=None, zbufs=3, xcbufs=4, tabbufs=8, tpbufs=3,
                   zpbufs=3):
    from contextlib import ExitStack, nullcontext

    nc = bacc.Bacc("TRN2")
    inp = nc.declare_dram_parameter("inp", [2 * F, N], bf16, isOutput=False)
    w0 = nc.declare_dram_parameter("w0", [128, NT0, 128], bf16, isOutput=False)
    w0h = nc.declare_dram_parameter("w0h", [64, 128], bf16, isOutput=False)
    w1 = nc.declare_dram_parameter("w1", [128, NBF, 128], bf16, isOutput=False)
    w2 = nc.declare_dram_parameter("w2", [128, NBF, 128], bf16, isOutput=False)
    w1q = nc.declare_dram_parameter("w1q", [128, NPO, 128], fp8, isOutput=False)
    w2q = nc.declare_dram_parameter("w2q", [128, NPO, 128], fp8, isOutput=False)
    b0 = nc.declare_dram_parameter("b0", [128, 1], f32, isOutput=False)
    b1 = nc.declare_dram_parameter("b1", [128, 1], f32, isOutput=False)
    b2 = nc.declare_dram_parameter("b2", [128, 1], f32, isOutput=False)
    # tables: layer-0 sym tiles (16 full in 2 GRP-slots + one 64-row tail),
    # layer-1/2 bf16 f-row slots and the fp8 Pool-route table (IN/8)
    tab0 = nc.declare_dram_parameter("tab0", [NCH, 2, 128, GRP, CH], bf16,
                                     isOutput=False)
    tab0h = nc.declare_dram_parameter("tab0h", [NCH, 64, CH], bf16,
                                      isOutput=False)
    tab1 = nc.declare_dram_parameter("tab1", [NCH, NSL, 128, GRP, CH], bf16,
                                     isOutput=False)
    tabp = nc.declare_dram_parameter("tabp", [NCH, 128, NPO, CH], fp8,
                                     isOutput=False)
    out = nc.declare_dram_parameter("out", [3, 128, BL], f32, isOutput=True)

    with tile.TileContext(nc) as tc, ExitStack() as ctx:
        wpool = ctx.enter_context(tc.tile_pool(name="w", bufs=1))
        xpool = ctx.enter_context(tc.tile_pool(name="x0", bufs=1))
        xc_pool = ctx.enter_context(tc.tile_pool(name="xc", bufs=xcbufs))
        tabs = ctx.enter_context(tc.tile_pool(name="tabs", bufs=tabbufs))
        tps = ctx.enter_context(tc.tile_pool(name="tps", bufs=tpbufs))
        zpool = ctx.enter_context(tc.tile_pool(name="z", bufs=zbufs))
        zppool = ctx.enter_context(tc.tile_pool(name="zp", bufs=zpbufs))
        opool = ctx.enter_context(tc.tile_pool(name="oacc", bufs=1))
        pspool = ctx.enter_context(tc.tile_pool(name="ps", bufs=3, space="PSUM"))

        # resident weights / constants
        w0_t = wpool.tile([128, NT0, 128], bf16)
        nc.sync.dma_start(w0_t[:], w0[:])
        w0h_t = wpool.tile([64, 128], bf16)
        nc.sync.dma_start(w0h_t[:], w0h[:])
        w1_t = wpool.tile([128, NBF, 128], bf16)
        nc.sync.dma_start(w1_t[:], w1[:])
        w2_t = wpool.tile([128, NBF, 128], bf16)
        nc.sync.dma_start(w2_t[:], w2[:])
        w1q_t = wpool.tile([128, NPO, 128], fp8)
        nc.sync.dma_start(w1q_t[:], w1q[:])
        w2q_t = wpool.tile([128, NPO, 128], fp8)
        nc.sync.dma_start(w2q_t[:], w2q[:])
        bias_ts = []
        for nm, bd in (("b0", b0), ("b1", b1), ("b2", b2)):
            bt = wpool.tile([128, 1], f32, name=nm)
            nc.sync.dma_start(bt[:], bd[:])
            bias_ts.append(bt)

        # X0 stacked twice: [IN; IN] so partition p holds IN[p mod 64]
        x0_t = xpool.tile([128, N], bf16)
        nc.sync.dma_start(x0_t[:], inp[:])

        oacc = [opool.tile([128, BL], f32, name=f"oacc{i}", tag=f"oacc{i}")
                for i in range(3)]

        loop_cm = tc.For_i(0, bench_repeat, 1) if bench_repeat else nullcontext()
        with loop_cm:
          for c in range(NCH):
            ns = c * CH
            bsl = c * (CH // D)

            # ---- stream tables for this chunk
            t0 = []
            for g in range(2):
                s = tabs.tile([128, GRP, CH], bf16, tag="tab")
                nc.sync.dma_start(s[:], tab0[c, g])
                t0.append(s)
            t0h = tabs.tile([64, CH], bf16, tag="tabh")
            nc.sync.dma_start(t0h[:], tab0h[c])
            t1 = []
            for g in range(NSL):
                s = tabs.tile([128, GRP, CH], bf16, tag="tab")
                nc.sync.dma_start(s[:], tab1[c, g])
                t1.append(s)
            tp = tps.tile([128, NPO, CH], fp8, tag="tp")
            nc.sync.dma_start(tp[:], tabp[c])

            # ---- layer 0: 16 sym k-tiles + one K=64 tail
            ps0 = pspool.tile([128, CH], f32, tag="ps")
            for g in range(2):
                z8 = zpool.tile([128, GRP, CH], bf16, tag="z")
                nc.vector.tensor_mul(
                    z8[:], x0_t[:, ns:ns + CH].unsqueeze(1)
                    .broadcast_to([128, GRP, CH]), t0[g][:])
                for j in range(GRP):
                    m = g * GRP + j
                    nc.tensor.matmul(ps0[:], w0_t[:, m, :], z8[:, j, :],
                                     start=(m == 0), stop=False)
            zh = zpool.tile([64, CH], bf16, tag="zh")
            nc.vector.tensor_mul(zh[:], x0_t[0:64, ns:ns + CH], t0h[:])
            nc.tensor.matmul(ps0[:], w0h_t[:], zh[:], start=False, stop=True)

            x1c = xc_pool.tile([128, CH], bf16, tag="xc")
            nc.scalar.activation(x1c[:], ps0[:],
                                 mybir.ActivationFunctionType.Relu,
                                 bias=bias_ts[0], scale=1.0)
            nc.vector.tensor_reduce(
                oacc[0][:, bsl:bsl + CH // D],
                x1c.rearrange("p (g d) -> p g d", d=D),
                axis=mybir.AxisListType.X, op=mybir.AluOpType.add)

            # ---- layers 1 and 2
            xin = x1c
            for li, (w_t, wq_t) in ((1, (w1_t, w1q_t)), (2, (w2_t, w2q_t))):
                ps = pspool.tile([128, CH], f32, tag="ps")

                # Pool route first so GPSIMD starts while DVE works
                zp = zppool.tile([128, NPO, CH], fp8, tag="zp")
                nc.gpsimd.tensor_mul(
                    zp[:], xin.unsqueeze(1).broadcast_to([128, NPO, CH]),
                    tp[:])

                for g in range(NSL):
                    z8 = zpool.tile([128, GRP, CH], bf16, tag="z")
                    nc.vector.tensor_mul(
                        z8[:], xin.unsqueeze(1)
                        .broadcast_to([128, GRP, CH]), t1[g][:])
                    for j in range(GRP):
                        f = g * GRP + j
                        nc.tensor.matmul(ps[:], w_t[:, f, :], z8[:, j, :],
                                         start=(f == 0), stop=False)
                for f in range(NPO):
                    nc.tensor.matmul(ps[:], wq_t[:, f, :], zp[:, f, :],
                                     start=False, stop=(f == NPO - 1))

                xo = xc_pool.tile([128, CH], bf16, tag="xc")
                nc.scalar.activation(xo[:], ps[:],
                                     mybir.ActivationFunctionType.Relu,
                                     bias=bias_ts[li], scale=1.0)
                nc.vector.tensor_reduce(
                    oacc[li][:, bsl:bsl + CH // D],
                    xo.rearrange("p (g d) -> p g d", d=D),
                    axis=mybir.AxisListType.X, op=mybir.AluOpType.add)
                xin = xo

          for li in range(3):
            nc.sync.dma_start(out[li], oacc[li][:])

    nc.finalize()
    return nc


def _pack_weights(W0, b0, W1, b1, W2, b2):
    O_, F_ = 128, 64
    W0r = np.asarray(W0, np.float32).reshape(O_, F_, F_)   # [o, h, f]
    SW0 = W0r + W0r.transpose(0, 2, 1)

    # layer 0: tile m (0..15) packs groups t=2m (p<64) and t=2m+1 (p>=64);
    # tail tile = group t=32 at half weight. weight[p, m, o].
    a = np.arange(64)
    w0p = np.empty((128, NT0, O_), np.float32)
    for m in range(NT0):
        for half, t in ((0, 2 * m), (1, 2 * m + 1)):
            f = (a + t) % 64
            wv = SW0[:, a, f]                    # [o, 64]
            if t == 0:
                wv = wv / 2                      # diag counted twice in SW0
            w0p[half * 64:half * 64 + 64, m, :] = wv.T
    fh = (a + 32) % 64
    w0h = (SW0[:, a, fh] / 2).T                  # [64, o]

    def pack_l(W):
        Wr = np.asarray(W, np.float32).reshape(O_, 128, F_)   # [o, h, f]
        Wp = Wr.transpose(1, 2, 0)                            # [h, f, o]
        wt = np.ascontiguousarray(Wp[:, :NBF, :]).astype(BF16)
        wq = np.ascontiguousarray(Wp[:, NBF:, :] * SW).astype(E4M3)
        return wt, wq

    w1p, w1qp = pack_l(W1)
    w2p, w2qp = pack_l(W2)

    return {
        "w0": w0p.astype(BF16), "w0h": w0h.astype(BF16),
        "w1": w1p, "w2": w2p, "w1q": w1qp, "w2q": w2qp,
        "b0": np.asarray(b0, np.float32).reshape(128, 1),
        "b1": np.asarray(b1, np.float32).reshape(128, 1),
        "b2": np.asarray(b2, np.float32).reshape(128, 1),
    }


def make_in_maps(input, W0, b0, W1, b1, W2, b2):
    shared = _pack_weights(W0, b0, W1, b1, W2, b2)
    a = np.arange(64)
    in_maps = []
    inp_np = np.asarray(input)
    for core in range(NCORES):
        shard = inp_np[core * BL:(core + 1) * BL]          # [BL, F, D]
        INf32 = np.ascontiguousarray(
            shard.transpose(1, 0, 2).reshape(F, N)).astype(np.float32)
        IN = INf32.astype(BF16)
        INs = np.ascontiguousarray(np.concatenate([IN, IN], axis=0))
        INfc = IN.reshape(F, NCH, CH)
        # layer-0 sym tables: tab0[c, g, p, j, n] = IN[(p%64 + t)%64, ...],
        # t = 2*(8g+j) + p//64
        t0a = np.empty((NCH, 2, 128, GRP, CH), BF16)
        for g in range(2):
            for j in range(GRP):
                m = g * GRP + j
                t0a[:, g, 0:64, j, :] = np.transpose(
                    INfc[(a + 2 * m) % 64], (1, 0, 2))
                t0a[:, g, 64:128, j, :] = np.transpose(
                    INfc[(a + 2 * m + 1) % 64], (1, 0, 2))
        t0h = np.ascontiguousarray(
            np.transpose(INfc[(a + 32) % 64], (1, 0, 2)))      # [NCH, 64, CH]
        # layer-1/2 bf16 tables for DVE-route f-slices (f 0..NBF-1)
        t1r = np.transpose(INfc[:NBF].reshape(NSL, GRP, NCH, CH), (2, 0, 1, 3))
        t1a = np.empty((NCH, NSL, 128, GRP, CH), BF16)
        t1a[:, :] = t1r[:, :, None, :, :]
        # Pool-route fp8 table: IN[f 48..63]/8, partition-replicated
        tpf = np.transpose(INfc[NBF:].astype(np.float32) / SW,
                           (1, 0, 2))                          # [NCH, NPO, CH]
        tpa = np.empty((NCH, 128, NPO, CH), E4M3)
        tpa[:, :] = tpf.astype(E4M3)[:, None, :, :]
        in_maps.append({"inp": INs, "tab0": t0a, "tab0h": t0h, "tab1": t1a,
                        "tabp": tpa, **shared})
    return in_maps


def gather_out(results):
    return np.concatenate(
        [np.asarray(r["out"], np.float32).transpose(2, 0, 1).reshape(BL, 3 * O)
         for r in results], axis=0)


def kernel(input, W0, b0, W1, b1, W2, b2):
    if "nc" not in _cache:
        _cache["nc"] = _build_program()
    nc = _cache["nc"]
    in_maps = make_in_maps(input, W0, b0, W1, b1, W2, b2)
    res = run_bass_kernel_spmd(nc, in_maps, list(range(NCORES)))
    return gather_out(res.results)
